# revision 5
# baseline (speedup 1.0000x reference)
"""Distributed kNN retrieval + subjective-logic fusion kernel for 8 Trainium2 cores.

Strategy (classic distributed kNN per the sharding hint):
  - Shard the memory bank across 8 cores along N (12500 rows each, zero-padded
    to 12800).
  - Each core computes cosine sims for all 1024 queries against its shard
    (bf16 matmul, fp32 PSUM accumulate) and extracts its local top-16
    candidates per query with a tagged-word scheme:
       word = (bf16(sim) << 16) | position         (uint32)
    Interpreted as fp32, word order == sim order (position = tiebreak), so
    every max/group-max op carries its argmax for free.
    Pipeline: PE matmul -> ACT writes bf16 sims into hi-halves of a
    position-tagged uint32 array -> DVE grouped reduce_max (groups of 32) ->
    top-16 groups via max8/match_replace/max8 -> spill tags to DRAM ->
    per-(query,group) indirect-DMA gather of the 16 winning groups ->
    final top-16-of-512 via max8/match_replace/max8 -> 16 tagged words/query.
  - Host merges the 8x16 candidates per query ("all-gather + global top-k"),
    rescores them with exact fp32 dot products (0.2% of the matmul FLOPs; makes
    selection and softmax values exact vs the fp32 reference), applies softmax
    and the Dirichlet/DST fusion.
"""
import sys
sys.path.insert(0, '/opt/trn_rl_repo')
from contextlib import ExitStack

import numpy as np
import ml_dtypes

import concourse.bass as bass
import concourse.tile as tile
from concourse import mybir, bacc, bass_utils

EPS = 1e-8
TEMPERATURE = 0.07

B, D, N, K = 1024, 256, 100000, 2
NCORES = 8
NLOC_REAL = N // NCORES          # 12500
NLOC = 12800                     # padded shard size (25 x 512)
L = 32                           # group size for the scan
G = NLOC // L                    # 400 groups
QT = 128                         # queries per tile
NQT = B // QT                    # 8 query tiles
SUB = 512                        # matmul moving chunk (one PSUM bank of fp32)
CHUNK = 2048                     # ACT copy / scan / spill chunk (4 PSUM banks)
TOPK = 16

_cache = {}


def _build_program():
    nc = bacc.Bacc("TRN2", target_bir_lowering=False, debug=False)

    mt = nc.dram_tensor("mt", [128, 2, NLOC], mybir.dt.bfloat16, kind="ExternalInput")
    qt = nc.dram_tensor("qt", [128, 2, B], mybir.dt.bfloat16, kind="ExternalInput")
    ow = nc.dram_tensor("ow", [B, TOPK], mybir.dt.uint32, kind="ExternalOutput")

    with tile.TileContext(nc) as tc, ExitStack() as ctx:
        const = ctx.enter_context(tc.tile_pool(name="const", bufs=1))
        small = ctx.enter_context(tc.tile_pool(name="small", bufs=2))
        psum = ctx.enter_context(tc.tile_pool(name="psum", bufs=2, space="PSUM"))
        dram = ctx.enter_context(tc.tile_pool(name="dram", bufs=1, space="DRAM"))

        mt_sb = const.tile([128, 2, NLOC], mybir.dt.bfloat16)
        nc.sync.dma_start(mt_sb[:], mt.ap())
        qt_sb = const.tile([128, 2, B], mybir.dt.bfloat16)
        nc.sync.dma_start(qt_sb[:], qt.ap())

        # two manually-alternated tagged-sims buffers, lo-halves iota'd once
        tts = []
        for i in range(2):
            tt_buf = const.tile([128, NLOC], mybir.dt.uint32, tag=f"tt{i}")
            tts.append(tt_buf)
        for t in tts:
            lo = t[:].bitcast(mybir.dt.uint16).rearrange("p (n two) -> p n two", two=2)[:, :, 0]
            nc.gpsimd.iota(lo, pattern=[[1, NLOC]], base=0, channel_multiplier=0)

        # spill as a table of groups: row r = (query q, group g), r = q*G + g
        spill = dram.tile([B * G, L], mybir.dt.uint32)
        spill2d = spill[:].rearrange("(q g) l -> q (g l)", g=G)

        cmask = const.tile([128, 16], mybir.dt.uint32)
        nc.vector.memset(cmask[:], 0x0000FFFF)
        cshift = const.tile([128, 16], mybir.dt.uint32)
        nc.vector.memset(cshift[:], 5)

        for t in range(NQT):
            tt = tts[t % 2]
            tt_bf = tt[:].bitcast(mybir.dt.bfloat16).rearrange("p (n two) -> p n two", two=2)[:, :, 1]
            tt_f32 = tt[:].bitcast(mybir.dt.float32)

            n_chunks = NLOC // CHUNK  # 6.25 -> handle 6 full + 1 partial inside
            c0 = 0
            ci = 0
            while c0 < NLOC:
                cl = min(CHUNK, NLOC - c0)
                ps = psum.tile([128, CHUNK], mybir.dt.float32)
                for s in range(0, cl, SUB):
                    for h in range(2):
                        nc.tensor.matmul(
                            ps[:, s:s + SUB],
                            qt_sb[:, h, t * QT:(t + 1) * QT],
                            mt_sb[:, h, c0 + s:c0 + s + SUB],
                            start=(h == 0), stop=(h == 1),
                        )
                # bf16 sims into hi-halves of the tagged words
                nc.scalar.copy(tt_bf[:, c0:c0 + cl], ps[:, :cl])
                # spill this chunk of tagged words to DRAM
                nc.sync.dma_start(
                    spill2d[t * QT:(t + 1) * QT, c0:c0 + cl],
                    tt[:, c0:c0 + cl],
                )
                c0 += cl
                ci += 1

            # grouped max scan over the whole row (tagged fp32 view)
            bm = small.tile([128, G], mybir.dt.float32, tag="bm")
            nc.vector.reduce_max(
                bm[:], tt_f32.rearrange("p (g l) -> p g l", l=L),
                axis=mybir.AxisListType.X,
            )

            # top-16 groups: max8 + match_replace + max8
            wins = small.tile([128, 16], mybir.dt.float32, tag="wins")
            bm2 = small.tile([128, G], mybir.dt.float32, tag="bm2")
            nc.vector.max(wins[:, 0:8], bm[:])
            nc.vector.match_replace(bm2[:], wins[:, 0:8], bm[:], -3.0e38)
            nc.vector.max(wins[:, 8:16], bm2[:])

            # gather row index: ((word & 0xFFFF) >> 5) + (t*128+p)*G
            wu = wins[:].bitcast(mybir.dt.uint32)
            offs = small.tile([128, 16], mybir.dt.uint32, tag="offs")
            nc.vector.tensor_tensor(offs[:], wu, cmask[:], mybir.AluOpType.bitwise_and)
            nc.vector.tensor_tensor(offs[:], offs[:], cshift[:],
                                    mybir.AluOpType.logical_shift_right)
            qbase = small.tile([128, 16], mybir.dt.uint32, tag="qbase")
            nc.gpsimd.iota(qbase[:], pattern=[[0, 16]], base=t * QT * G,
                           channel_multiplier=G)
            nc.vector.tensor_tensor(offs[:], offs[:], qbase[:],
                                    mybir.AluOpType.add)

            # gather the 16 winning groups (32 tagged words each) from DRAM
            ic = small.tile([128, 16, L], mybir.dt.uint32, tag="ic")
            for j in range(16):
                nc.gpsimd.indirect_dma_start(
                    out=ic[:, j, :],
                    out_offset=None,
                    in_=spill[:],
                    in_offset=bass.IndirectOffsetOnAxis(ap=offs[:, j:j + 1], axis=0),
                )

            # final top-16 of the gathered 512 tagged words
            icf = ic[:].rearrange("p a b -> p (a b)").bitcast(mybir.dt.float32)
            fin = small.tile([128, 16], mybir.dt.float32, tag="fin")
            ic2 = small.tile([128, 16 * L], mybir.dt.float32, tag="ic2")
            nc.vector.max(fin[:, 0:8], icf)
            nc.vector.match_replace(ic2[:], fin[:, 0:8], icf, -3.0e38)
            nc.vector.max(fin[:, 8:16], ic2[:])

            nc.sync.dma_start(ow.ap()[t * QT:(t + 1) * QT, :],
                              fin[:].bitcast(mybir.dt.uint32))

    nc.compile()
    return nc


def _get_program():
    if "nc" not in _cache:
        _cache["nc"] = _build_program()
    return _cache["nc"]


def _prep_inputs(query, memory_feat):
    qn = np.sqrt((query.astype(np.float32) ** 2).sum(-1, keepdims=True))
    qhat = query / np.clip(qn, EPS, None)
    mn = np.sqrt((memory_feat.astype(np.float32) ** 2).sum(-1, keepdims=True))
    mhat = memory_feat / np.clip(mn, EPS, None)

    # qt: (128, 2, B) bf16 with qt[p, h, b] = qhat[b, h*128+p]
    qtl = np.ascontiguousarray(
        qhat.T.reshape(2, 128, B).transpose(1, 0, 2)
    ).astype(ml_dtypes.bfloat16)

    # memory shards: (128, 2, NLOC) bf16 with mt[p, h, j] = mhat[core*12500 + j, h*128+p]
    mts = []
    for c in range(NCORES):
        slab = mhat[c * NLOC_REAL:(c + 1) * NLOC_REAL]
        slab = np.concatenate(
            [slab, np.zeros((NLOC - NLOC_REAL, D), np.float32)], axis=0
        )
        mtl = np.ascontiguousarray(
            slab.T.reshape(2, 128, NLOC).transpose(1, 0, 2)
        ).astype(ml_dtypes.bfloat16)
        mts.append(mtl)
    return qhat, mhat, qtl, mts


def _fuse_host(topv, topi, memory_evidence, model_evidence):
    """Exact fp32 mirror of the reference softmax + DST fusion."""
    f32 = np.float32
    w = topv.astype(f32) / f32(TEMPERATURE)
    w = w - w.max(-1, keepdims=True)
    w = np.exp(w)
    w = w / w.sum(-1, keepdims=True)

    ev = memory_evidence[topi]                      # (B, k, K)
    alpha_r = f32(1.0) + np.einsum("bk,bkc->bc", w, ev.astype(f32))
    alpha_m = model_evidence.astype(f32) + f32(1.0)

    def alpha_to_belief_u(alpha):
        Kd = alpha.shape[-1]
        S = np.clip(alpha.sum(-1, keepdims=True), EPS, None)
        b = np.clip((alpha - 1.0) / S, 0.0, None)
        u = np.clip(Kd / S, EPS, 1.0 - EPS)
        b_sum = b.sum(-1, keepdims=True)
        target = np.clip(1.0 - u, EPS, None)
        b = b * (target / np.clip(b_sum, EPS, None))
        return b.astype(f32), u.astype(f32)

    def combine_two_opinions(b1, u1, b2, u2):
        total_pair = b1.sum(-1, keepdims=True) * b2.sum(-1, keepdims=True)
        dot_same = (b1 * b2).sum(-1, keepdims=True)
        C = total_pair - dot_same
        S = np.clip(1.0 - C, EPS, None)
        b = (b1 * b2 + b1 * u2 + b2 * u1) / S
        u = u1 * u2 / S
        b = np.clip(b, 0.0, None)
        u = np.clip(u, EPS, 1.0 - EPS)
        b_sum = b.sum(-1, keepdims=True)
        b = b * ((1.0 - u) / np.clip(b_sum, EPS, None))
        return b.astype(f32), u.astype(f32)

    def opinion_to_alpha(b, u):
        Kd = b.shape[-1]
        u = np.clip(u, EPS, 1.0 - EPS)
        S = Kd / u
        alpha = b * S + 1.0
        return np.clip(alpha, 1.0 + EPS, None).astype(f32)

    b_m, u_m = alpha_to_belief_u(alpha_m)
    b_r, u_r = alpha_to_belief_u(alpha_r)
    b_f, u_f = combine_two_opinions(b_m, u_m, b_r, u_r)
    return opinion_to_alpha(b_f, u_f)


def kernel(query, memory_feat, memory_evidence, model_evidence, top_k):
    top_k = int(top_k)
    assert top_k == TOPK

    query = np.asarray(query, dtype=np.float32)
    memory_feat = np.asarray(memory_feat, dtype=np.float32)
    memory_evidence = np.asarray(memory_evidence, dtype=np.float32)
    model_evidence = np.asarray(model_evidence, dtype=np.float32)

    nc = _get_program()
    qhat, mhat, qtl, mts = _prep_inputs(query, memory_feat)

    in_maps = [{"mt": mts[c], "qt": qtl} for c in range(NCORES)]
    res = bass_utils.run_bass_kernel_spmd(nc, in_maps, core_ids=list(range(NCORES)))
    _cache["last_results"] = res

    # decode candidates: pos = word & 0xFFFF (position within the core's slab)
    cand_idx = np.empty((B, NCORES * TOPK), dtype=np.int64)
    for c in range(NCORES):
        words = res.results[c]["ow"]                 # (B, 16) uint32
        pos = (words & 0xFFFF).astype(np.int64)
        valid = pos < NLOC_REAL
        gidx = c * NLOC_REAL + np.clip(pos, 0, NLOC_REAL - 1)
        gidx[~valid] = -1
        cand_idx[:, c * TOPK:(c + 1) * TOPK] = gidx

    # exact fp32 rescore of the 128 candidates per query
    safe_idx = np.clip(cand_idx, 0, N - 1)
    mh_c = mhat[safe_idx]                            # (B, 128, D)
    s = np.einsum("bd,bkd->bk", qhat, mh_c).astype(np.float32)
    s[cand_idx < 0] = -np.inf
    # dedupe not needed: positions are distinct per core and cores are disjoint

    order = np.argsort(-s, axis=1, kind="stable")[:, :TOPK]
    topv = np.take_along_axis(s, order, axis=1)
    topi = np.take_along_axis(cand_idx, order, axis=1)

    return _fuse_host(topv, topi, memory_evidence, model_evidence)


# revision 13
# speedup vs baseline: 2.7538x; 2.7538x over previous
"""Distributed kNN retrieval + subjective-logic fusion kernel for 8 Trainium2 cores.

Strategy (classic distributed kNN per the sharding hint):
  - Shard the memory bank across 8 cores along N (12500 rows each, zero-padded
    to 12800).  Host prepares normalized, transposed bf16 operand layouts
    (layout/dtype prep only; all O(B*N*D) compute runs on device).
  - Each core computes cosine sims for all 1024 queries against its shard
    (bf16 matmul, fp32 PSUM) and selects its local top-16 candidates/query:
      PE matmul -> ACT copies PSUM to a bf16 sims plane -> DVE grouped
      reduce_max (groups of 32) -> top-16 groups via max8/max_index/
      match_replace -> spill sims plane to DRAM -> per-(query,group)
      indirect-DMA gather of the 16 winning groups -> top-16-of-512 via
      max8/max_index -> outputs two index arrays (group ids + positions).
  - Host composes the two index levels into global candidate indices
    ("all-gather the M*k candidates"), rescores the 8x16 candidates per query
    with exact fp32 dot products (0.2% of the matmul FLOPs; makes selection
    and softmax exactly match the fp32 reference), then applies softmax and
    the Dirichlet/DST opinion fusion.
"""
import sys
sys.path.insert(0, '/opt/trn_rl_repo')
from contextlib import ExitStack

import numpy as np
import ml_dtypes

import concourse.bass as bass
import concourse.tile as tile
from concourse import mybir, bacc, bass_utils

EPS = 1e-8
TEMPERATURE = 0.07

B, D, N, K = 1024, 256, 100000, 2
NCORES = 8
NLOC_REAL = N // NCORES          # 12500
NLOC = 12800                     # padded shard size
L = 32                           # group size for the scan
G = NLOC // L                    # 400 groups per query row
QT = 128                         # queries per tile
NQT = B // QT                    # 8 query tiles
SUB = 512                        # matmul moving chunk (one PSUM fp32 bank)
CHUNK = 2048                     # PSUM tile / copy / scan / spill chunk
TOPK = 16

_cache = {}


def _build_program(repeat=1):
    nc = bacc.Bacc("TRN2", target_bir_lowering=False, debug=False)

    mt = nc.dram_tensor("mt", [128, 2, NLOC], mybir.dt.bfloat16, kind="ExternalInput")
    qt = nc.dram_tensor("qt", [128, 2, B], mybir.dt.bfloat16, kind="ExternalInput")
    og = nc.dram_tensor("og", [B, TOPK], mybir.dt.uint32, kind="ExternalOutput")
    of = nc.dram_tensor("of", [B, TOPK], mybir.dt.uint32, kind="ExternalOutput")

    with tile.TileContext(nc) as tc, ExitStack() as ctx:
        const = ctx.enter_context(tc.tile_pool(name="const", bufs=1))
        small = ctx.enter_context(tc.tile_pool(name="small", bufs=4))
        psum = ctx.enter_context(tc.tile_pool(name="psum", bufs=2, space="PSUM"))
        dram = ctx.enter_context(tc.tile_pool(name="dram", bufs=1, space="DRAM"))

        qt_sb = const.tile([128, 2, B], mybir.dt.bfloat16)
        nc.sync.dma_start(qt_sb[:], qt.ap())
        # chunked memory load so the first matmuls start early
        mt_sb = const.tile([128, 2, NLOC], mybir.dt.bfloat16)
        for c0 in range(0, NLOC, CHUNK):
            cl = min(CHUNK, NLOC - c0)
            nc.sync.dma_start(mt_sb[:, :, c0:c0 + cl], mt.ap()[:, :, c0:c0 + cl])

        # bf16 sims planes, manually triple-buffered across q-tiles
        NSIMS = 3
        sims = []
        for i in range(NSIMS):
            sims_buf = const.tile([128, NLOC], mybir.dt.bfloat16, tag=f"sims{i}")
            sims.append(sims_buf)

        # gather row base: p*G, same for every q-tile
        qbase = const.tile([128, 16], mybir.dt.uint32)
        nc.gpsimd.iota(qbase[:], pattern=[[0, 16]], base=0, channel_multiplier=G)

        # one spill tensor per q-tile (avoids WAR serialization between the
        # indirect gathers of tile t and the spill DMAs of tile t+1)
        spills = []
        for t in range(NQT):
            spill_buf = dram.tile([QT * G, L], mybir.dt.bfloat16, tag=f"spill{t}")
            spills.append(spill_buf)

        for t in [tq for _ in range(repeat) for tq in range(NQT)]:
            sb = sims[t % NSIMS]
            spill = spills[t]

            bm = small.tile([128, G], mybir.dt.bfloat16, tag="bm")
            c0 = 0
            while c0 < NLOC:
                cl = min(CHUNK, NLOC - c0)
                ps = psum.tile([128, CHUNK], mybir.dt.float32)
                for s in range(0, cl, SUB):
                    for h in range(2):
                        nc.tensor.matmul(
                            ps[:, s:s + SUB],
                            qt_sb[:, h, t * QT:(t + 1) * QT],
                            mt_sb[:, h, c0 + s:c0 + s + SUB],
                            start=(h == 0), stop=(h == 1),
                        )
                # PSUM -> bf16 sims plane (contiguous)
                nc.scalar.copy(sb[:, c0:c0 + cl], ps[:, :cl])
                # spill this chunk to DRAM (row q*G+g of L bf16)
                nc.sync.dma_start(
                    spill[:].rearrange("r l -> (r l)").rearrange(
                        "(q n) -> q n", q=QT)[:, c0:c0 + cl],
                    sb[:, c0:c0 + cl],
                )
                # grouped max scan of this chunk
                nc.vector.reduce_max(
                    bm[:, c0 // L:(c0 + cl) // L],
                    sb[:, c0:c0 + cl].rearrange("p (g l) -> p g l", l=L),
                    axis=mybir.AxisListType.X,
                )
                c0 += cl

            # top-16 groups: values + ids
            gv = small.tile([128, 16], mybir.dt.bfloat16, tag="gv")
            gi = small.tile([128, 16], mybir.dt.uint32, tag="gi")
            bm2 = small.tile([128, G], mybir.dt.bfloat16, tag="bm2")
            nc.vector.max(gv[:, 0:8], bm[:])
            nc.vector.max_index(gi[:, 0:8], gv[:, 0:8], bm[:])
            nc.vector.match_replace(bm2[:], gv[:, 0:8], bm[:], -3.0e38)
            nc.vector.max(gv[:, 8:16], bm2[:])
            nc.vector.max_index(gi[:, 8:16], gv[:, 8:16], bm2[:])

            # gather row index: gi + p*G  (row q*G+g within this tile's spill)
            offs = small.tile([128, 16], mybir.dt.uint32, tag="offs")
            nc.vector.tensor_tensor(offs[:], gi[:], qbase[:], mybir.AluOpType.add)

            # gather the 16 winning groups (L bf16 words each) from DRAM
            ic = small.tile([128, 16, L], mybir.dt.bfloat16, tag="ic")
            for j in range(16):
                nc.gpsimd.indirect_dma_start(
                    out=ic[:, j, :],
                    out_offset=None,
                    in_=spill[:],
                    in_offset=bass.IndirectOffsetOnAxis(ap=offs[:, j:j + 1], axis=0),
                )

            # final top-16 of the gathered 512
            icf = ic[:].rearrange("p a b -> p (a b)")
            fv = small.tile([128, 16], mybir.dt.bfloat16, tag="fv")
            fi = small.tile([128, 16], mybir.dt.uint32, tag="fi")
            ic2 = small.tile([128, 16 * L], mybir.dt.bfloat16, tag="ic2")
            nc.vector.max(fv[:, 0:8], icf)
            nc.vector.max_index(fi[:, 0:8], fv[:, 0:8], icf)
            nc.vector.match_replace(ic2[:], fv[:, 0:8], icf, -3.0e38)
            nc.vector.max(fv[:, 8:16], ic2[:])
            nc.vector.max_index(fi[:, 8:16], fv[:, 8:16], ic2[:])

            nc.sync.dma_start(og.ap()[t * QT:(t + 1) * QT, :], gi[:])
            nc.sync.dma_start(of.ap()[t * QT:(t + 1) * QT, :], fi[:])

    nc.compile()
    return nc


def _get_program():
    if "nc" not in _cache:
        _cache["nc"] = _build_program()
    return _cache["nc"]


def _prep_inputs(query, memory_feat):
    qn = np.sqrt((query.astype(np.float32) ** 2).sum(-1, keepdims=True))
    qhat = query / np.clip(qn, EPS, None)
    mn = np.sqrt((memory_feat.astype(np.float32) ** 2).sum(-1, keepdims=True))
    mhat = memory_feat / np.clip(mn, EPS, None)

    # qt: (128, 2, B) bf16 with qt[p, h, b] = qhat[b, h*128+p]
    qtl = np.ascontiguousarray(
        qhat.T.reshape(2, 128, B).transpose(1, 0, 2)
    ).astype(ml_dtypes.bfloat16)

    # memory shards: (128, 2, NLOC) bf16 with mt[p, h, j] = mhat[c*12500+j, h*128+p]
    mts = []
    for c in range(NCORES):
        slab = mhat[c * NLOC_REAL:(c + 1) * NLOC_REAL]
        slab = np.concatenate(
            [slab, np.zeros((NLOC - NLOC_REAL, D), np.float32)], axis=0
        )
        mtl = np.ascontiguousarray(
            slab.T.reshape(2, 128, NLOC).transpose(1, 0, 2)
        ).astype(ml_dtypes.bfloat16)
        mts.append(mtl)
    return qhat, mhat, qtl, mts


def _fuse_host(topv, topi, memory_evidence, model_evidence):
    """Exact fp32 mirror of the reference softmax + DST fusion."""
    f32 = np.float32
    w = topv.astype(f32) / f32(TEMPERATURE)
    w = w - w.max(-1, keepdims=True)
    w = np.exp(w)
    w = w / w.sum(-1, keepdims=True)

    ev = memory_evidence[topi]                      # (B, k, K)
    alpha_r = f32(1.0) + np.einsum("bk,bkc->bc", w, ev.astype(f32))
    alpha_m = model_evidence.astype(f32) + f32(1.0)

    def alpha_to_belief_u(alpha):
        Kd = alpha.shape[-1]
        S = np.clip(alpha.sum(-1, keepdims=True), EPS, None)
        b = np.clip((alpha - 1.0) / S, 0.0, None)
        u = np.clip(Kd / S, EPS, 1.0 - EPS)
        b_sum = b.sum(-1, keepdims=True)
        target = np.clip(1.0 - u, EPS, None)
        b = b * (target / np.clip(b_sum, EPS, None))
        return b.astype(f32), u.astype(f32)

    def combine_two_opinions(b1, u1, b2, u2):
        total_pair = b1.sum(-1, keepdims=True) * b2.sum(-1, keepdims=True)
        dot_same = (b1 * b2).sum(-1, keepdims=True)
        C = total_pair - dot_same
        S = np.clip(1.0 - C, EPS, None)
        b = (b1 * b2 + b1 * u2 + b2 * u1) / S
        u = u1 * u2 / S
        b = np.clip(b, 0.0, None)
        u = np.clip(u, EPS, 1.0 - EPS)
        b_sum = b.sum(-1, keepdims=True)
        b = b * ((1.0 - u) / np.clip(b_sum, EPS, None))
        return b.astype(f32), u.astype(f32)

    def opinion_to_alpha(b, u):
        Kd = b.shape[-1]
        u = np.clip(u, EPS, 1.0 - EPS)
        S = Kd / u
        alpha = b * S + 1.0
        return np.clip(alpha, 1.0 + EPS, None).astype(f32)

    b_m, u_m = alpha_to_belief_u(alpha_m)
    b_r, u_r = alpha_to_belief_u(alpha_r)
    b_f, u_f = combine_two_opinions(b_m, u_m, b_r, u_r)
    return opinion_to_alpha(b_f, u_f)


def kernel(query, memory_feat, memory_evidence, model_evidence, top_k):
    top_k = int(top_k)
    assert top_k == TOPK

    query = np.asarray(query, dtype=np.float32)
    memory_feat = np.asarray(memory_feat, dtype=np.float32)
    memory_evidence = np.asarray(memory_evidence, dtype=np.float32)
    model_evidence = np.asarray(model_evidence, dtype=np.float32)

    nc = _get_program()
    qhat, mhat, qtl, mts = _prep_inputs(query, memory_feat)

    in_maps = [{"mt": mts[c], "qt": qtl} for c in range(NCORES)]
    res = bass_utils.run_bass_kernel_spmd(nc, in_maps, core_ids=list(range(NCORES)))
    _cache["last_results"] = res

    # compose the two index levels: global candidate index per (core, q, slot)
    cand_idx = np.empty((B, NCORES * TOPK), dtype=np.int64)
    for c in range(NCORES):
        gids = res.results[c]["og"].astype(np.int64)    # (B,16) group ids in [0,G)
        fidx = res.results[c]["of"].astype(np.int64)    # (B,16) positions in [0,512)
        j = fidx >> 5                                    # which gathered slot
        r = fidx & 31                                    # position within group
        grp = np.take_along_axis(gids, j, axis=1)        # group id per candidate
        pos = grp * L + r                                # position in the slab
        valid = pos < NLOC_REAL
        gidx = c * NLOC_REAL + np.clip(pos, 0, NLOC_REAL - 1)
        gidx[~valid] = -1
        cand_idx[:, c * TOPK:(c + 1) * TOPK] = gidx

    # exact fp32 rescore of the 128 candidates per query
    safe_idx = np.clip(cand_idx, 0, N - 1)
    mh_c = mhat[safe_idx]                                # (B, 128, D)
    s = np.einsum("bd,bkd->bk", qhat, mh_c).astype(np.float32)
    s[cand_idx < 0] = -np.inf

    order = np.argsort(-s, axis=1, kind="stable")[:, :TOPK]
    topv = np.take_along_axis(s, order, axis=1)
    topi = np.take_along_axis(cand_idx, order, axis=1)

    return _fuse_host(topv, topi, memory_evidence, model_evidence)


# revision 15
# speedup vs baseline: 2.9858x; 1.0843x over previous
"""Distributed kNN retrieval + subjective-logic fusion kernel for 8 Trainium2 cores.

Strategy (classic distributed kNN per the sharding hint):
  - Shard the memory bank across 8 cores along N (12500 rows each, zero-padded
    to 12800).  Host prepares normalized, transposed bf16 operand layouts
    (layout/dtype prep only; all O(B*N*D) compute runs on device).
  - Each core computes cosine sims for all 1024 queries against its shard
    (bf16 matmul, fp32 PSUM) and selects its local top-16 candidates/query:
      PE matmul -> ACT copies PSUM to a bf16 sims plane -> DVE grouped
      reduce_max (groups of 32) -> top-16 groups via max8/max_index/
      match_replace -> spill sims plane to DRAM -> per-(query,group)
      indirect-DMA gather of the 16 winning groups -> top-16-of-512 via
      max8/max_index -> outputs two index arrays (group ids + positions).
  - Host composes the two index levels into global candidate indices
    ("all-gather the M*k candidates"), rescores the 8x16 candidates per query
    with exact fp32 dot products (0.2% of the matmul FLOPs; makes selection
    and softmax exactly match the fp32 reference), then applies softmax and
    the Dirichlet/DST opinion fusion.
"""
import sys
sys.path.insert(0, '/opt/trn_rl_repo')
from contextlib import ExitStack

import numpy as np
import ml_dtypes

import concourse.bass as bass
import concourse.tile as tile
from concourse import mybir, bacc, bass_utils

EPS = 1e-8
TEMPERATURE = 0.07

B, D, N, K = 1024, 256, 100000, 2
NCORES = 8
NLOC_REAL = N // NCORES          # 12500
NLOC = 12800                     # padded shard size
L = 32                           # group size for the scan
G = NLOC // L                    # 400 groups per query row
QT = 128                         # queries per tile
NQT = B // QT                    # 8 query tiles
SUB = 512                        # matmul moving chunk (one PSUM fp32 bank)
CHUNK = 2048                     # PSUM tile / copy / scan / spill chunk
TOPK = 16

_cache = {}


def _build_program(repeat=1):
    nc = bacc.Bacc("TRN2", target_bir_lowering=False, debug=False)

    mt = nc.dram_tensor("mt", [128, 2, NLOC], mybir.dt.bfloat16, kind="ExternalInput")
    qt = nc.dram_tensor("qt", [128, 2, B], mybir.dt.bfloat16, kind="ExternalInput")
    og = nc.dram_tensor("og", [B, TOPK], mybir.dt.uint32, kind="ExternalOutput")
    of = nc.dram_tensor("of", [B, TOPK], mybir.dt.uint32, kind="ExternalOutput")

    with tile.TileContext(nc) as tc, ExitStack() as ctx:
        const = ctx.enter_context(tc.tile_pool(name="const", bufs=1))
        small = ctx.enter_context(tc.tile_pool(name="small", bufs=6))
        psum = ctx.enter_context(tc.tile_pool(name="psum", bufs=2, space="PSUM"))
        dram = ctx.enter_context(tc.tile_pool(name="dram", bufs=1, space="DRAM"))

        qt_sb = const.tile([128, 2, B], mybir.dt.bfloat16)
        nc.sync.dma_start(qt_sb[:], qt.ap())
        # chunked memory load so the first matmuls start early
        mt_sb = const.tile([128, 2, NLOC], mybir.dt.bfloat16)
        for c0 in range(0, NLOC, CHUNK):
            cl = min(CHUNK, NLOC - c0)
            nc.sync.dma_start(mt_sb[:, :, c0:c0 + cl], mt.ap()[:, :, c0:c0 + cl])

        # bf16 sims planes, manually triple-buffered across q-tiles
        NSIMS = 4
        sims = []
        for i in range(NSIMS):
            sims_buf = const.tile([128, NLOC], mybir.dt.bfloat16, tag=f"sims{i}")
            sims.append(sims_buf)

        # gather row base: p*G, same for every q-tile
        qbase = const.tile([128, 16], mybir.dt.uint32)
        nc.gpsimd.iota(qbase[:], pattern=[[0, 16]], base=0, channel_multiplier=G)

        # one spill tensor per q-tile (avoids WAR serialization between the
        # indirect gathers of tile t and the spill DMAs of tile t+1)
        spills = []
        for t in range(NQT):
            spill_buf = dram.tile([QT * G, L], mybir.dt.bfloat16, tag=f"spill{t}")
            spills.append(spill_buf)

        for t in [tq for _ in range(repeat) for tq in range(NQT)]:
            sb = sims[t % NSIMS]
            spill = spills[t]

            bm = small.tile([128, G], mybir.dt.bfloat16, tag="bm")
            c0 = 0
            while c0 < NLOC:
                cl = min(CHUNK, NLOC - c0)
                ps = psum.tile([128, CHUNK], mybir.dt.float32)
                for s in range(0, cl, SUB):
                    for h in range(2):
                        nc.tensor.matmul(
                            ps[:, s:s + SUB],
                            qt_sb[:, h, t * QT:(t + 1) * QT],
                            mt_sb[:, h, c0 + s:c0 + s + SUB],
                            start=(h == 0), stop=(h == 1),
                        )
                # PSUM -> bf16 sims plane (contiguous)
                nc.scalar.copy(sb[:, c0:c0 + cl], ps[:, :cl])
                # spill this chunk to DRAM (row q*G+g of L bf16)
                nc.sync.dma_start(
                    spill[:].rearrange("r l -> (r l)").rearrange(
                        "(q n) -> q n", q=QT)[:, c0:c0 + cl],
                    sb[:, c0:c0 + cl],
                )
                # grouped max scan of this chunk
                nc.vector.reduce_max(
                    bm[:, c0 // L:(c0 + cl) // L],
                    sb[:, c0:c0 + cl].rearrange("p (g l) -> p g l", l=L),
                    axis=mybir.AxisListType.X,
                )
                c0 += cl

            # top-16 groups (two rounds of 8); gathers for round 1 dispatch
            # while round 2 still runs on the vector engine
            gv = small.tile([128, 16], mybir.dt.bfloat16, tag="gv")
            gi = small.tile([128, 16], mybir.dt.uint32, tag="gi")
            bm2 = small.tile([128, G], mybir.dt.bfloat16, tag="bm2")
            offs = small.tile([128, 16], mybir.dt.uint32, tag="offs")
            ic = small.tile([128, 16, L], mybir.dt.bfloat16, tag="ic")

            nc.vector.max(gv[:, 0:8], bm[:])
            nc.vector.max_index(gi[:, 0:8], gv[:, 0:8], bm[:])
            nc.vector.tensor_tensor(offs[:, 0:8], gi[:, 0:8], qbase[:, 0:8],
                                    mybir.AluOpType.add)
            for j in range(8):
                nc.gpsimd.indirect_dma_start(
                    out=ic[:, j, :], out_offset=None, in_=spill[:],
                    in_offset=bass.IndirectOffsetOnAxis(ap=offs[:, j:j + 1], axis=0),
                )

            nc.vector.match_replace(bm2[:], gv[:, 0:8], bm[:], -3.0e38)
            nc.vector.max(gv[:, 8:16], bm2[:])
            nc.vector.max_index(gi[:, 8:16], gv[:, 8:16], bm2[:])
            nc.vector.tensor_tensor(offs[:, 8:16], gi[:, 8:16], qbase[:, 8:16],
                                    mybir.AluOpType.add)
            for j in range(8, 16):
                nc.gpsimd.indirect_dma_start(
                    out=ic[:, j, :], out_offset=None, in_=spill[:],
                    in_offset=bass.IndirectOffsetOnAxis(ap=offs[:, j:j + 1], axis=0),
                )

            # final top-16 of the gathered 512
            icf = ic[:].rearrange("p a b -> p (a b)")
            fv = small.tile([128, 16], mybir.dt.bfloat16, tag="fv")
            fi = small.tile([128, 16], mybir.dt.uint32, tag="fi")
            ic2 = small.tile([128, 16 * L], mybir.dt.bfloat16, tag="ic2")
            nc.vector.max(fv[:, 0:8], icf)
            nc.vector.max_index(fi[:, 0:8], fv[:, 0:8], icf)
            nc.vector.match_replace(ic2[:], fv[:, 0:8], icf, -3.0e38)
            nc.vector.max(fv[:, 8:16], ic2[:])
            nc.vector.max_index(fi[:, 8:16], fv[:, 8:16], ic2[:])

            nc.sync.dma_start(og.ap()[t * QT:(t + 1) * QT, :], gi[:])
            nc.sync.dma_start(of.ap()[t * QT:(t + 1) * QT, :], fi[:])

    nc.compile()
    return nc


def _get_program():
    if "nc" not in _cache:
        _cache["nc"] = _build_program()
    return _cache["nc"]


def _prep_inputs(query, memory_feat):
    qn = np.sqrt((query.astype(np.float32) ** 2).sum(-1, keepdims=True))
    qhat = query / np.clip(qn, EPS, None)
    mn = np.sqrt((memory_feat.astype(np.float32) ** 2).sum(-1, keepdims=True))
    mhat = memory_feat / np.clip(mn, EPS, None)

    # qt: (128, 2, B) bf16 with qt[p, h, b] = qhat[b, h*128+p]
    qtl = np.ascontiguousarray(
        qhat.T.reshape(2, 128, B).transpose(1, 0, 2)
    ).astype(ml_dtypes.bfloat16)

    # memory shards: (128, 2, NLOC) bf16 with mt[p, h, j] = mhat[c*12500+j, h*128+p]
    mts = []
    for c in range(NCORES):
        slab = mhat[c * NLOC_REAL:(c + 1) * NLOC_REAL]
        slab = np.concatenate(
            [slab, np.zeros((NLOC - NLOC_REAL, D), np.float32)], axis=0
        )
        mtl = np.ascontiguousarray(
            slab.T.reshape(2, 128, NLOC).transpose(1, 0, 2)
        ).astype(ml_dtypes.bfloat16)
        mts.append(mtl)
    return qhat, mhat, qtl, mts


def _fuse_host(topv, topi, memory_evidence, model_evidence):
    """Exact fp32 mirror of the reference softmax + DST fusion."""
    f32 = np.float32
    w = topv.astype(f32) / f32(TEMPERATURE)
    w = w - w.max(-1, keepdims=True)
    w = np.exp(w)
    w = w / w.sum(-1, keepdims=True)

    ev = memory_evidence[topi]                      # (B, k, K)
    alpha_r = f32(1.0) + np.einsum("bk,bkc->bc", w, ev.astype(f32))
    alpha_m = model_evidence.astype(f32) + f32(1.0)

    def alpha_to_belief_u(alpha):
        Kd = alpha.shape[-1]
        S = np.clip(alpha.sum(-1, keepdims=True), EPS, None)
        b = np.clip((alpha - 1.0) / S, 0.0, None)
        u = np.clip(Kd / S, EPS, 1.0 - EPS)
        b_sum = b.sum(-1, keepdims=True)
        target = np.clip(1.0 - u, EPS, None)
        b = b * (target / np.clip(b_sum, EPS, None))
        return b.astype(f32), u.astype(f32)

    def combine_two_opinions(b1, u1, b2, u2):
        total_pair = b1.sum(-1, keepdims=True) * b2.sum(-1, keepdims=True)
        dot_same = (b1 * b2).sum(-1, keepdims=True)
        C = total_pair - dot_same
        S = np.clip(1.0 - C, EPS, None)
        b = (b1 * b2 + b1 * u2 + b2 * u1) / S
        u = u1 * u2 / S
        b = np.clip(b, 0.0, None)
        u = np.clip(u, EPS, 1.0 - EPS)
        b_sum = b.sum(-1, keepdims=True)
        b = b * ((1.0 - u) / np.clip(b_sum, EPS, None))
        return b.astype(f32), u.astype(f32)

    def opinion_to_alpha(b, u):
        Kd = b.shape[-1]
        u = np.clip(u, EPS, 1.0 - EPS)
        S = Kd / u
        alpha = b * S + 1.0
        return np.clip(alpha, 1.0 + EPS, None).astype(f32)

    b_m, u_m = alpha_to_belief_u(alpha_m)
    b_r, u_r = alpha_to_belief_u(alpha_r)
    b_f, u_f = combine_two_opinions(b_m, u_m, b_r, u_r)
    return opinion_to_alpha(b_f, u_f)


def kernel(query, memory_feat, memory_evidence, model_evidence, top_k):
    top_k = int(top_k)
    assert top_k == TOPK

    query = np.asarray(query, dtype=np.float32)
    memory_feat = np.asarray(memory_feat, dtype=np.float32)
    memory_evidence = np.asarray(memory_evidence, dtype=np.float32)
    model_evidence = np.asarray(model_evidence, dtype=np.float32)

    nc = _get_program()
    qhat, mhat, qtl, mts = _prep_inputs(query, memory_feat)

    in_maps = [{"mt": mts[c], "qt": qtl} for c in range(NCORES)]
    res = bass_utils.run_bass_kernel_spmd(nc, in_maps, core_ids=list(range(NCORES)))
    _cache["last_results"] = res

    # compose the two index levels: global candidate index per (core, q, slot)
    cand_idx = np.empty((B, NCORES * TOPK), dtype=np.int64)
    for c in range(NCORES):
        gids = res.results[c]["og"].astype(np.int64)    # (B,16) group ids in [0,G)
        fidx = res.results[c]["of"].astype(np.int64)    # (B,16) positions in [0,512)
        j = fidx >> 5                                    # which gathered slot
        r = fidx & 31                                    # position within group
        grp = np.take_along_axis(gids, j, axis=1)        # group id per candidate
        pos = grp * L + r                                # position in the slab
        valid = pos < NLOC_REAL
        gidx = c * NLOC_REAL + np.clip(pos, 0, NLOC_REAL - 1)
        gidx[~valid] = -1
        cand_idx[:, c * TOPK:(c + 1) * TOPK] = gidx

    # exact fp32 rescore of the 128 candidates per query
    safe_idx = np.clip(cand_idx, 0, N - 1)
    mh_c = mhat[safe_idx]                                # (B, 128, D)
    s = np.einsum("bd,bkd->bk", qhat, mh_c).astype(np.float32)
    s[cand_idx < 0] = -np.inf

    order = np.argsort(-s, axis=1, kind="stable")[:, :TOPK]
    topv = np.take_along_axis(s, order, axis=1)
    topi = np.take_along_axis(cand_idx, order, axis=1)

    return _fuse_host(topv, topi, memory_evidence, model_evidence)


# revision 17
# speedup vs baseline: 3.1261x; 1.0470x over previous
"""Distributed kNN retrieval + subjective-logic fusion kernel for 8 Trainium2 cores.

Strategy (classic distributed kNN per the sharding hint):
  - Shard the memory bank across 8 cores along N (12500 rows each, zero-padded
    to 12800).  Host prepares normalized, transposed bf16 operand layouts
    (layout/dtype prep only; all O(B*N*D) compute runs on device).
  - Each core computes cosine sims for all 1024 queries against its shard
    (bf16 matmul, fp32 PSUM) and selects its local top-16 candidates/query:
      PE matmul -> ACT copies PSUM to a bf16 sims plane -> DVE grouped
      reduce_max (groups of 32) -> top-16 groups via max8/max_index/
      match_replace -> spill sims plane to DRAM -> per-(query,group)
      indirect-DMA gather of the 16 winning groups -> top-16-of-512 via
      max8/max_index -> outputs two index arrays (group ids + positions).
  - Host composes the two index levels into global candidate indices
    ("all-gather the M*k candidates"), rescores the 8x16 candidates per query
    with exact fp32 dot products (0.2% of the matmul FLOPs; makes selection
    and softmax exactly match the fp32 reference), then applies softmax and
    the Dirichlet/DST opinion fusion.
"""
import sys
sys.path.insert(0, '/opt/trn_rl_repo')
from contextlib import ExitStack

import numpy as np
import ml_dtypes

import concourse.bass as bass
import concourse.tile as tile
from concourse import mybir, bacc, bass_utils

EPS = 1e-8
TEMPERATURE = 0.07

B, D, N, K = 1024, 256, 100000, 2
NCORES = 8
NLOC_REAL = N // NCORES          # 12500
NLOC = 12800                     # padded shard size
L = 32                           # group size for the scan
G = NLOC // L                    # 400 groups per query row
QT = 128                         # queries per tile
NQT = B // QT                    # 8 query tiles
SUB = 512                        # matmul moving chunk (one PSUM fp32 bank)
CHUNK = 2048                     # PSUM tile / copy / scan / spill chunk
TOPK = 16

_cache = {}


def _build_program(repeat=1):
    nc = bacc.Bacc("TRN2", target_bir_lowering=False, debug=False)

    mt = nc.dram_tensor("mt", [128, 2, NLOC], mybir.dt.bfloat16, kind="ExternalInput")
    qt = nc.dram_tensor("qt", [128, 2, B], mybir.dt.bfloat16, kind="ExternalInput")
    og = nc.dram_tensor("og", [B, TOPK], mybir.dt.uint32, kind="ExternalOutput")
    of = nc.dram_tensor("of", [B, TOPK], mybir.dt.uint32, kind="ExternalOutput")

    with tile.TileContext(nc) as tc, ExitStack() as ctx:
        const = ctx.enter_context(tc.tile_pool(name="const", bufs=1))
        small = ctx.enter_context(tc.tile_pool(name="small", bufs=6))
        psum = ctx.enter_context(tc.tile_pool(name="psum", bufs=2, space="PSUM"))
        dram = ctx.enter_context(tc.tile_pool(name="dram", bufs=1, space="DRAM"))

        qt_sb = const.tile([128, 2, B], mybir.dt.bfloat16)
        nc.sync.dma_start(qt_sb[:], qt.ap())
        # chunked memory load so the first matmuls start early
        mt_sb = const.tile([128, 2, NLOC], mybir.dt.bfloat16)
        for c0 in range(0, NLOC, CHUNK):
            cl = min(CHUNK, NLOC - c0)
            nc.sync.dma_start(mt_sb[:, :, c0:c0 + cl], mt.ap()[:, :, c0:c0 + cl])

        # bf16 sims planes, manually triple-buffered across q-tiles
        NSIMS = 4
        sims = []
        for i in range(NSIMS):
            sims_buf = const.tile([128, NLOC], mybir.dt.bfloat16, tag=f"sims{i}")
            sims.append(sims_buf)

        # gather row base: p*G, same for every q-tile
        qbase = const.tile([128, 16], mybir.dt.uint32)
        nc.gpsimd.iota(qbase[:], pattern=[[0, 16]], base=0, channel_multiplier=G)

        # one spill tensor per q-tile (avoids WAR serialization between the
        # indirect gathers of tile t and the spill DMAs of tile t+1)
        spills = []
        for t in range(NQT):
            spill_buf = dram.tile([QT * G, L], mybir.dt.bfloat16, tag=f"spill{t}")
            spills.append(spill_buf)

        pending = []

        def _emit_final(item):
            pt, pic, pgi = item
            picf = pic[:].rearrange("p a b -> p (a b)")
            fv = small.tile([128, 16], mybir.dt.bfloat16, tag="fv")
            fi = small.tile([128, 16], mybir.dt.uint32, tag="fi")
            ic2 = small.tile([128, 16 * L], mybir.dt.bfloat16, tag="ic2")
            nc.vector.max(fv[:, 0:8], picf)
            nc.vector.max_index(fi[:, 0:8], fv[:, 0:8], picf)
            nc.vector.match_replace(ic2[:], fv[:, 0:8], picf, -3.0e38)
            nc.vector.max(fv[:, 8:16], ic2[:])
            nc.vector.max_index(fi[:, 8:16], fv[:, 8:16], ic2[:])
            nc.sync.dma_start(og.ap()[pt * QT:(pt + 1) * QT, :], pgi[:])
            nc.sync.dma_start(of.ap()[pt * QT:(pt + 1) * QT, :], fi[:])

        for t in [tq for _ in range(repeat) for tq in range(NQT)]:
            sb = sims[t % NSIMS]
            spill = spills[t]

            bm = small.tile([128, G], mybir.dt.bfloat16, tag="bm")
            c0 = 0
            while c0 < NLOC:
                cl = min(CHUNK, NLOC - c0)
                ps = psum.tile([128, CHUNK], mybir.dt.float32)
                for s in range(0, cl, SUB):
                    for h in range(2):
                        nc.tensor.matmul(
                            ps[:, s:s + SUB],
                            qt_sb[:, h, t * QT:(t + 1) * QT],
                            mt_sb[:, h, c0 + s:c0 + s + SUB],
                            start=(h == 0), stop=(h == 1),
                        )
                # PSUM -> bf16 sims plane (contiguous)
                nc.scalar.copy(sb[:, c0:c0 + cl], ps[:, :cl])
                # spill this chunk to DRAM (row q*G+g of L bf16)
                nc.sync.dma_start(
                    spill[:].rearrange("r l -> (r l)").rearrange(
                        "(q n) -> q n", q=QT)[:, c0:c0 + cl],
                    sb[:, c0:c0 + cl],
                )
                # grouped max scan of this chunk
                nc.vector.reduce_max(
                    bm[:, c0 // L:(c0 + cl) // L],
                    sb[:, c0:c0 + cl].rearrange("p (g l) -> p g l", l=L),
                    axis=mybir.AxisListType.X,
                )
                c0 += cl

            # top-16 groups (two rounds of 8); gathers for round 1 dispatch
            # while round 2 still runs on the vector engine
            gv = small.tile([128, 16], mybir.dt.bfloat16, tag="gv")
            gi = small.tile([128, 16], mybir.dt.uint32, tag="gi")
            bm2 = small.tile([128, G], mybir.dt.bfloat16, tag="bm2")
            offs = small.tile([128, 16], mybir.dt.uint32, tag="offs")
            ic = small.tile([128, 16, L], mybir.dt.bfloat16, tag="ic")

            nc.vector.max(gv[:, 0:8], bm[:])
            nc.vector.max_index(gi[:, 0:8], gv[:, 0:8], bm[:])
            nc.vector.tensor_tensor(offs[:, 0:8], gi[:, 0:8], qbase[:, 0:8],
                                    mybir.AluOpType.add)
            for j in range(8):
                nc.gpsimd.indirect_dma_start(
                    out=ic[:, j, :], out_offset=None, in_=spill[:],
                    in_offset=bass.IndirectOffsetOnAxis(ap=offs[:, j:j + 1], axis=0),
                )

            nc.vector.match_replace(bm2[:], gv[:, 0:8], bm[:], -3.0e38)
            nc.vector.max(gv[:, 8:16], bm2[:])
            nc.vector.max_index(gi[:, 8:16], gv[:, 8:16], bm2[:])
            nc.vector.tensor_tensor(offs[:, 8:16], gi[:, 8:16], qbase[:, 8:16],
                                    mybir.AluOpType.add)
            for j in range(8, 16):
                nc.gpsimd.indirect_dma_start(
                    out=ic[:, j, :], out_offset=None, in_=spill[:],
                    in_offset=bass.IndirectOffsetOnAxis(ap=offs[:, j:j + 1], axis=0),
                )

            # defer the final extraction of this tile until after the next
            # tile's matmul/scan/gather section is emitted (software
            # pipelining: gives the scheduler better interleave priorities)
            pending.append((t, ic, gi))
            if len(pending) > 1:
                _emit_final(pending.pop(0))

        for p in pending:
            _emit_final(p)

    nc.compile()
    return nc


def _get_program():
    if "nc" not in _cache:
        _cache["nc"] = _build_program()
    return _cache["nc"]


def _prep_inputs(query, memory_feat):
    qn = np.sqrt((query.astype(np.float32) ** 2).sum(-1, keepdims=True))
    qhat = query / np.clip(qn, EPS, None)
    mn = np.sqrt((memory_feat.astype(np.float32) ** 2).sum(-1, keepdims=True))
    mhat = memory_feat / np.clip(mn, EPS, None)

    # qt: (128, 2, B) bf16 with qt[p, h, b] = qhat[b, h*128+p]
    qtl = np.ascontiguousarray(
        qhat.T.reshape(2, 128, B).transpose(1, 0, 2)
    ).astype(ml_dtypes.bfloat16)

    # memory shards: (128, 2, NLOC) bf16 with mt[p, h, j] = mhat[c*12500+j, h*128+p]
    mts = []
    for c in range(NCORES):
        slab = mhat[c * NLOC_REAL:(c + 1) * NLOC_REAL]
        slab = np.concatenate(
            [slab, np.zeros((NLOC - NLOC_REAL, D), np.float32)], axis=0
        )
        mtl = np.ascontiguousarray(
            slab.T.reshape(2, 128, NLOC).transpose(1, 0, 2)
        ).astype(ml_dtypes.bfloat16)
        mts.append(mtl)
    return qhat, mhat, qtl, mts


def _fuse_host(topv, topi, memory_evidence, model_evidence):
    """Exact fp32 mirror of the reference softmax + DST fusion."""
    f32 = np.float32
    w = topv.astype(f32) / f32(TEMPERATURE)
    w = w - w.max(-1, keepdims=True)
    w = np.exp(w)
    w = w / w.sum(-1, keepdims=True)

    ev = memory_evidence[topi]                      # (B, k, K)
    alpha_r = f32(1.0) + np.einsum("bk,bkc->bc", w, ev.astype(f32))
    alpha_m = model_evidence.astype(f32) + f32(1.0)

    def alpha_to_belief_u(alpha):
        Kd = alpha.shape[-1]
        S = np.clip(alpha.sum(-1, keepdims=True), EPS, None)
        b = np.clip((alpha - 1.0) / S, 0.0, None)
        u = np.clip(Kd / S, EPS, 1.0 - EPS)
        b_sum = b.sum(-1, keepdims=True)
        target = np.clip(1.0 - u, EPS, None)
        b = b * (target / np.clip(b_sum, EPS, None))
        return b.astype(f32), u.astype(f32)

    def combine_two_opinions(b1, u1, b2, u2):
        total_pair = b1.sum(-1, keepdims=True) * b2.sum(-1, keepdims=True)
        dot_same = (b1 * b2).sum(-1, keepdims=True)
        C = total_pair - dot_same
        S = np.clip(1.0 - C, EPS, None)
        b = (b1 * b2 + b1 * u2 + b2 * u1) / S
        u = u1 * u2 / S
        b = np.clip(b, 0.0, None)
        u = np.clip(u, EPS, 1.0 - EPS)
        b_sum = b.sum(-1, keepdims=True)
        b = b * ((1.0 - u) / np.clip(b_sum, EPS, None))
        return b.astype(f32), u.astype(f32)

    def opinion_to_alpha(b, u):
        Kd = b.shape[-1]
        u = np.clip(u, EPS, 1.0 - EPS)
        S = Kd / u
        alpha = b * S + 1.0
        return np.clip(alpha, 1.0 + EPS, None).astype(f32)

    b_m, u_m = alpha_to_belief_u(alpha_m)
    b_r, u_r = alpha_to_belief_u(alpha_r)
    b_f, u_f = combine_two_opinions(b_m, u_m, b_r, u_r)
    return opinion_to_alpha(b_f, u_f)


def kernel(query, memory_feat, memory_evidence, model_evidence, top_k):
    top_k = int(top_k)
    assert top_k == TOPK

    query = np.asarray(query, dtype=np.float32)
    memory_feat = np.asarray(memory_feat, dtype=np.float32)
    memory_evidence = np.asarray(memory_evidence, dtype=np.float32)
    model_evidence = np.asarray(model_evidence, dtype=np.float32)

    nc = _get_program()
    qhat, mhat, qtl, mts = _prep_inputs(query, memory_feat)

    in_maps = [{"mt": mts[c], "qt": qtl} for c in range(NCORES)]
    res = bass_utils.run_bass_kernel_spmd(nc, in_maps, core_ids=list(range(NCORES)))
    _cache["last_results"] = res

    # compose the two index levels: global candidate index per (core, q, slot)
    cand_idx = np.empty((B, NCORES * TOPK), dtype=np.int64)
    for c in range(NCORES):
        gids = res.results[c]["og"].astype(np.int64)    # (B,16) group ids in [0,G)
        fidx = res.results[c]["of"].astype(np.int64)    # (B,16) positions in [0,512)
        j = fidx >> 5                                    # which gathered slot
        r = fidx & 31                                    # position within group
        grp = np.take_along_axis(gids, j, axis=1)        # group id per candidate
        pos = grp * L + r                                # position in the slab
        valid = pos < NLOC_REAL
        gidx = c * NLOC_REAL + np.clip(pos, 0, NLOC_REAL - 1)
        gidx[~valid] = -1
        cand_idx[:, c * TOPK:(c + 1) * TOPK] = gidx

    # exact fp32 rescore of the 128 candidates per query
    safe_idx = np.clip(cand_idx, 0, N - 1)
    mh_c = mhat[safe_idx]                                # (B, 128, D)
    s = np.einsum("bd,bkd->bk", qhat, mh_c).astype(np.float32)
    s[cand_idx < 0] = -np.inf

    order = np.argsort(-s, axis=1, kind="stable")[:, :TOPK]
    topv = np.take_along_axis(s, order, axis=1)
    topi = np.take_along_axis(cand_idx, order, axis=1)

    return _fuse_host(topv, topi, memory_evidence, model_evidence)


# revision 18
# speedup vs baseline: 3.1888x; 1.0200x over previous
"""Distributed kNN retrieval + subjective-logic fusion kernel for 8 Trainium2 cores.

Strategy (classic distributed kNN per the sharding hint):
  - Shard the memory bank across 8 cores along N (12500 rows each, zero-padded
    to 12800).  Host prepares normalized, transposed bf16 operand layouts
    (layout/dtype prep only; all O(B*N*D) compute runs on device).
  - Each core computes cosine sims for all 1024 queries against its shard
    (bf16 matmul, fp32 PSUM) and selects its local top-16 candidates/query:
      PE matmul -> ACT copies PSUM to a bf16 sims plane -> DVE grouped
      reduce_max (groups of 32) -> top-16 groups via max8/max_index/
      match_replace -> spill sims plane to DRAM -> per-(query,group)
      indirect-DMA gather of the 16 winning groups -> top-16-of-512 via
      max8/max_index -> outputs two index arrays (group ids + positions).
  - Host composes the two index levels into global candidate indices
    ("all-gather the M*k candidates"), rescores the 8x16 candidates per query
    with exact fp32 dot products (0.2% of the matmul FLOPs; makes selection
    and softmax exactly match the fp32 reference), then applies softmax and
    the Dirichlet/DST opinion fusion.
"""
import sys
sys.path.insert(0, '/opt/trn_rl_repo')
from contextlib import ExitStack

import numpy as np
import ml_dtypes

import concourse.bass as bass
import concourse.tile as tile
from concourse import mybir, bacc, bass_utils

EPS = 1e-8
TEMPERATURE = 0.07

B, D, N, K = 1024, 256, 100000, 2
NCORES = 8
NLOC_REAL = N // NCORES          # 12500
NLOC = 12800                     # padded shard size
L = 32                           # group size for the scan
G = NLOC // L                    # 400 groups per query row
QT = 128                         # queries per tile
NQT = B // QT                    # 8 query tiles
SUB = 512                        # matmul moving chunk (one PSUM fp32 bank)
CHUNK = 1024                     # PSUM tile / copy / scan / spill chunk
TOPK = 16

_cache = {}


def _build_program(repeat=1):
    nc = bacc.Bacc("TRN2", target_bir_lowering=False, debug=False)

    mt = nc.dram_tensor("mt", [128, 2, NLOC], mybir.dt.bfloat16, kind="ExternalInput")
    qt = nc.dram_tensor("qt", [128, 2, B], mybir.dt.bfloat16, kind="ExternalInput")
    og = nc.dram_tensor("og", [B, TOPK], mybir.dt.uint32, kind="ExternalOutput")
    of = nc.dram_tensor("of", [B, TOPK], mybir.dt.uint32, kind="ExternalOutput")

    with tile.TileContext(nc) as tc, ExitStack() as ctx:
        const = ctx.enter_context(tc.tile_pool(name="const", bufs=1))
        small = ctx.enter_context(tc.tile_pool(name="small", bufs=6))
        psum = ctx.enter_context(tc.tile_pool(name="psum", bufs=4, space="PSUM"))
        dram = ctx.enter_context(tc.tile_pool(name="dram", bufs=1, space="DRAM"))

        qt_sb = const.tile([128, 2, B], mybir.dt.bfloat16)
        nc.gpsimd.dma_start(qt_sb[:], qt.ap())
        # chunked memory load so the first matmuls start early
        mt_sb = const.tile([128, 2, NLOC], mybir.dt.bfloat16)
        for c0 in range(0, NLOC, CHUNK):
            cl = min(CHUNK, NLOC - c0)
            nc.sync.dma_start(mt_sb[:, :, c0:c0 + cl], mt.ap()[:, :, c0:c0 + cl])

        # bf16 sims planes, manually triple-buffered across q-tiles
        NSIMS = 4
        sims = []
        for i in range(NSIMS):
            sims_buf = const.tile([128, NLOC], mybir.dt.bfloat16, tag=f"sims{i}")
            sims.append(sims_buf)

        # gather row base: p*G, same for every q-tile
        qbase = const.tile([128, 16], mybir.dt.uint32)
        nc.gpsimd.iota(qbase[:], pattern=[[0, 16]], base=0, channel_multiplier=G)

        # one spill tensor per q-tile (avoids WAR serialization between the
        # indirect gathers of tile t and the spill DMAs of tile t+1)
        spills = []
        for t in range(NQT):
            spill_buf = dram.tile([QT * G, L], mybir.dt.bfloat16, tag=f"spill{t}")
            spills.append(spill_buf)

        pending = []

        def _emit_final(item):
            pt, pic, pgi = item
            picf = pic[:].rearrange("p a b -> p (a b)")
            fv = small.tile([128, 16], mybir.dt.bfloat16, tag="fv")
            fi = small.tile([128, 16], mybir.dt.uint32, tag="fi")
            ic2 = small.tile([128, 16 * L], mybir.dt.bfloat16, tag="ic2")
            nc.vector.max(fv[:, 0:8], picf)
            nc.vector.max_index(fi[:, 0:8], fv[:, 0:8], picf)
            nc.vector.match_replace(ic2[:], fv[:, 0:8], picf, -3.0e38)
            nc.vector.max(fv[:, 8:16], ic2[:])
            nc.vector.max_index(fi[:, 8:16], fv[:, 8:16], ic2[:])
            nc.sync.dma_start(og.ap()[pt * QT:(pt + 1) * QT, :], pgi[:])
            nc.sync.dma_start(of.ap()[pt * QT:(pt + 1) * QT, :], fi[:])

        for t in [tq for _ in range(repeat) for tq in range(NQT)]:
            sb = sims[t % NSIMS]
            spill = spills[t]

            bm = small.tile([128, G], mybir.dt.bfloat16, tag="bm")
            c0 = 0
            while c0 < NLOC:
                cl = min(CHUNK, NLOC - c0)
                ps = psum.tile([128, CHUNK], mybir.dt.float32)
                for s in range(0, cl, SUB):
                    for h in range(2):
                        nc.tensor.matmul(
                            ps[:, s:s + SUB],
                            qt_sb[:, h, t * QT:(t + 1) * QT],
                            mt_sb[:, h, c0 + s:c0 + s + SUB],
                            start=(h == 0), stop=(h == 1),
                        )
                # PSUM -> bf16 sims plane (contiguous)
                nc.scalar.copy(sb[:, c0:c0 + cl], ps[:, :cl])
                # spill this chunk to DRAM (row q*G+g of L bf16)
                nc.sync.dma_start(
                    spill[:].rearrange("r l -> (r l)").rearrange(
                        "(q n) -> q n", q=QT)[:, c0:c0 + cl],
                    sb[:, c0:c0 + cl],
                )
                # grouped max scan of this chunk
                nc.vector.reduce_max(
                    bm[:, c0 // L:(c0 + cl) // L],
                    sb[:, c0:c0 + cl].rearrange("p (g l) -> p g l", l=L),
                    axis=mybir.AxisListType.X,
                )
                c0 += cl

            # top-16 groups (two rounds of 8); gathers for round 1 dispatch
            # while round 2 still runs on the vector engine
            gv = small.tile([128, 16], mybir.dt.bfloat16, tag="gv")
            gi = small.tile([128, 16], mybir.dt.uint32, tag="gi")
            bm2 = small.tile([128, G], mybir.dt.bfloat16, tag="bm2")
            offs = small.tile([128, 16], mybir.dt.uint32, tag="offs")
            ic = small.tile([128, 16, L], mybir.dt.bfloat16, tag="ic")

            nc.vector.max(gv[:, 0:8], bm[:])
            nc.vector.max_index(gi[:, 0:8], gv[:, 0:8], bm[:])
            nc.vector.tensor_tensor(offs[:, 0:8], gi[:, 0:8], qbase[:, 0:8],
                                    mybir.AluOpType.add)
            for j in range(8):
                nc.gpsimd.indirect_dma_start(
                    out=ic[:, j, :], out_offset=None, in_=spill[:],
                    in_offset=bass.IndirectOffsetOnAxis(ap=offs[:, j:j + 1], axis=0),
                )

            nc.vector.match_replace(bm2[:], gv[:, 0:8], bm[:], -3.0e38)
            nc.vector.max(gv[:, 8:16], bm2[:])
            nc.vector.max_index(gi[:, 8:16], gv[:, 8:16], bm2[:])
            nc.vector.tensor_tensor(offs[:, 8:16], gi[:, 8:16], qbase[:, 8:16],
                                    mybir.AluOpType.add)
            for j in range(8, 16):
                nc.gpsimd.indirect_dma_start(
                    out=ic[:, j, :], out_offset=None, in_=spill[:],
                    in_offset=bass.IndirectOffsetOnAxis(ap=offs[:, j:j + 1], axis=0),
                )

            # defer the final extraction of this tile until after the next
            # tile's matmul/scan/gather section is emitted (software
            # pipelining: gives the scheduler better interleave priorities)
            pending.append((t, ic, gi))
            if len(pending) > 1:
                _emit_final(pending.pop(0))

        for p in pending:
            _emit_final(p)

    nc.compile()
    return nc


def _get_program():
    if "nc" not in _cache:
        _cache["nc"] = _build_program()
    return _cache["nc"]


def _prep_inputs(query, memory_feat):
    qn = np.sqrt((query.astype(np.float32) ** 2).sum(-1, keepdims=True))
    qhat = query / np.clip(qn, EPS, None)
    mn = np.sqrt((memory_feat.astype(np.float32) ** 2).sum(-1, keepdims=True))
    mhat = memory_feat / np.clip(mn, EPS, None)

    # qt: (128, 2, B) bf16 with qt[p, h, b] = qhat[b, h*128+p]
    qtl = np.ascontiguousarray(
        qhat.T.reshape(2, 128, B).transpose(1, 0, 2)
    ).astype(ml_dtypes.bfloat16)

    # memory shards: (128, 2, NLOC) bf16 with mt[p, h, j] = mhat[c*12500+j, h*128+p]
    mts = []
    for c in range(NCORES):
        slab = mhat[c * NLOC_REAL:(c + 1) * NLOC_REAL]
        slab = np.concatenate(
            [slab, np.zeros((NLOC - NLOC_REAL, D), np.float32)], axis=0
        )
        mtl = np.ascontiguousarray(
            slab.T.reshape(2, 128, NLOC).transpose(1, 0, 2)
        ).astype(ml_dtypes.bfloat16)
        mts.append(mtl)
    return qhat, mhat, qtl, mts


def _fuse_host(topv, topi, memory_evidence, model_evidence):
    """Exact fp32 mirror of the reference softmax + DST fusion."""
    f32 = np.float32
    w = topv.astype(f32) / f32(TEMPERATURE)
    w = w - w.max(-1, keepdims=True)
    w = np.exp(w)
    w = w / w.sum(-1, keepdims=True)

    ev = memory_evidence[topi]                      # (B, k, K)
    alpha_r = f32(1.0) + np.einsum("bk,bkc->bc", w, ev.astype(f32))
    alpha_m = model_evidence.astype(f32) + f32(1.0)

    def alpha_to_belief_u(alpha):
        Kd = alpha.shape[-1]
        S = np.clip(alpha.sum(-1, keepdims=True), EPS, None)
        b = np.clip((alpha - 1.0) / S, 0.0, None)
        u = np.clip(Kd / S, EPS, 1.0 - EPS)
        b_sum = b.sum(-1, keepdims=True)
        target = np.clip(1.0 - u, EPS, None)
        b = b * (target / np.clip(b_sum, EPS, None))
        return b.astype(f32), u.astype(f32)

    def combine_two_opinions(b1, u1, b2, u2):
        total_pair = b1.sum(-1, keepdims=True) * b2.sum(-1, keepdims=True)
        dot_same = (b1 * b2).sum(-1, keepdims=True)
        C = total_pair - dot_same
        S = np.clip(1.0 - C, EPS, None)
        b = (b1 * b2 + b1 * u2 + b2 * u1) / S
        u = u1 * u2 / S
        b = np.clip(b, 0.0, None)
        u = np.clip(u, EPS, 1.0 - EPS)
        b_sum = b.sum(-1, keepdims=True)
        b = b * ((1.0 - u) / np.clip(b_sum, EPS, None))
        return b.astype(f32), u.astype(f32)

    def opinion_to_alpha(b, u):
        Kd = b.shape[-1]
        u = np.clip(u, EPS, 1.0 - EPS)
        S = Kd / u
        alpha = b * S + 1.0
        return np.clip(alpha, 1.0 + EPS, None).astype(f32)

    b_m, u_m = alpha_to_belief_u(alpha_m)
    b_r, u_r = alpha_to_belief_u(alpha_r)
    b_f, u_f = combine_two_opinions(b_m, u_m, b_r, u_r)
    return opinion_to_alpha(b_f, u_f)


def kernel(query, memory_feat, memory_evidence, model_evidence, top_k):
    top_k = int(top_k)
    assert top_k == TOPK

    query = np.asarray(query, dtype=np.float32)
    memory_feat = np.asarray(memory_feat, dtype=np.float32)
    memory_evidence = np.asarray(memory_evidence, dtype=np.float32)
    model_evidence = np.asarray(model_evidence, dtype=np.float32)

    nc = _get_program()
    qhat, mhat, qtl, mts = _prep_inputs(query, memory_feat)

    in_maps = [{"mt": mts[c], "qt": qtl} for c in range(NCORES)]
    res = bass_utils.run_bass_kernel_spmd(nc, in_maps, core_ids=list(range(NCORES)))
    _cache["last_results"] = res

    # compose the two index levels: global candidate index per (core, q, slot)
    cand_idx = np.empty((B, NCORES * TOPK), dtype=np.int64)
    for c in range(NCORES):
        gids = res.results[c]["og"].astype(np.int64)    # (B,16) group ids in [0,G)
        fidx = res.results[c]["of"].astype(np.int64)    # (B,16) positions in [0,512)
        j = fidx >> 5                                    # which gathered slot
        r = fidx & 31                                    # position within group
        grp = np.take_along_axis(gids, j, axis=1)        # group id per candidate
        pos = grp * L + r                                # position in the slab
        valid = pos < NLOC_REAL
        gidx = c * NLOC_REAL + np.clip(pos, 0, NLOC_REAL - 1)
        gidx[~valid] = -1
        cand_idx[:, c * TOPK:(c + 1) * TOPK] = gidx

    # exact fp32 rescore of the 128 candidates per query
    safe_idx = np.clip(cand_idx, 0, N - 1)
    mh_c = mhat[safe_idx]                                # (B, 128, D)
    s = np.einsum("bd,bkd->bk", qhat, mh_c).astype(np.float32)
    s[cand_idx < 0] = -np.inf

    order = np.argsort(-s, axis=1, kind="stable")[:, :TOPK]
    topv = np.take_along_axis(s, order, axis=1)
    topi = np.take_along_axis(cand_idx, order, axis=1)

    return _fuse_host(topv, topi, memory_evidence, model_evidence)


# revision 20
# speedup vs baseline: 3.3726x; 1.0576x over previous
"""Distributed kNN retrieval + subjective-logic fusion kernel for 8 Trainium2 cores.

Strategy (classic distributed kNN per the sharding hint):
  - Shard the memory bank across 8 cores along N (12500 rows each, zero-padded
    to 12800).  Host prepares normalized, transposed bf16 operand layouts
    (layout/dtype prep only; all O(B*N*D) compute runs on device).
  - Each core computes cosine sims for all 1024 queries against its shard
    (bf16 matmul, fp32 PSUM) and selects its local top-16 candidates/query:
      PE matmul -> ACT copies PSUM to a bf16 sims plane -> DVE grouped
      reduce_max (groups of 32) -> top-16 groups via max8/max_index/
      match_replace -> spill sims plane to DRAM -> per-(query,group)
      indirect-DMA gather of the 16 winning groups -> top-16-of-512 via
      max8/max_index -> outputs two index arrays (group ids + positions).
  - Host composes the two index levels into global candidate indices
    ("all-gather the M*k candidates"), rescores the 8x16 candidates per query
    with exact fp32 dot products (0.2% of the matmul FLOPs; makes selection
    and softmax exactly match the fp32 reference), then applies softmax and
    the Dirichlet/DST opinion fusion.
"""
import sys
sys.path.insert(0, '/opt/trn_rl_repo')
from contextlib import ExitStack

import numpy as np
import ml_dtypes

import concourse.bass as bass
import concourse.tile as tile
from concourse import mybir, bacc, bass_utils

EPS = 1e-8
TEMPERATURE = 0.07

B, D, N, K = 1024, 256, 100000, 2
NCORES = 8
NLOC_REAL = N // NCORES          # 12500
NLOC = 12800                     # padded shard size
L = 32                           # group size for the scan
G = NLOC // L                    # 400 groups per query row
QT = 128                         # queries per tile
NQT = B // QT                    # 8 query tiles
SUB = 512                        # matmul moving chunk (one PSUM fp32 bank)
CHUNK = 1024                     # PSUM tile / copy / scan / spill chunk
TOPK = 16

_cache = {}


def _build_program(repeat=1):
    nc = bacc.Bacc("TRN2", target_bir_lowering=False, debug=False)

    mt = nc.dram_tensor("mt", [128, 2, NLOC], mybir.dt.bfloat16, kind="ExternalInput")
    qt = nc.dram_tensor("qt", [128, 2, B], mybir.dt.bfloat16, kind="ExternalInput")
    og = nc.dram_tensor("og", [B, TOPK], mybir.dt.uint32, kind="ExternalOutput")
    of = nc.dram_tensor("of", [B, TOPK], mybir.dt.uint32, kind="ExternalOutput")

    with tile.TileContext(nc) as tc, ExitStack() as ctx:
        const = ctx.enter_context(tc.tile_pool(name="const", bufs=1))
        small = ctx.enter_context(tc.tile_pool(name="small", bufs=6))
        psum = ctx.enter_context(tc.tile_pool(name="psum", bufs=4, space="PSUM"))
        dram = ctx.enter_context(tc.tile_pool(name="dram", bufs=1, space="DRAM"))

        qt_sb = const.tile([128, 2, B], mybir.dt.bfloat16)
        nc.gpsimd.dma_start(qt_sb[:], qt.ap())
        # chunked memory load so the first matmuls start early
        mt_sb = const.tile([128, 2, NLOC], mybir.dt.bfloat16)
        for c0 in range(0, NLOC, CHUNK):
            cl = min(CHUNK, NLOC - c0)
            nc.sync.dma_start(mt_sb[:, :, c0:c0 + cl], mt.ap()[:, :, c0:c0 + cl])

        # bf16 sims planes, manually triple-buffered across q-tiles
        NSIMS = 4
        sims = []
        for i in range(NSIMS):
            sims_buf = const.tile([128, NLOC], mybir.dt.bfloat16, tag=f"sims{i}")
            sims.append(sims_buf)

        # gather row base: p*G, same for every q-tile
        qbase = const.tile([128, 16], mybir.dt.uint32)
        nc.gpsimd.iota(qbase[:], pattern=[[0, 16]], base=0, channel_multiplier=G)

        # one spill tensor per q-tile (avoids WAR serialization between the
        # indirect gathers of tile t and the spill DMAs of tile t+1)
        spills = []
        for t in range(NQT):
            spill_buf = dram.tile([QT * G, L], mybir.dt.bfloat16, tag=f"spill{t}")
            spills.append(spill_buf)

        pending = []

        def _emit_final(item):
            pt, pic, pgi = item
            picf = pic[:].rearrange("p a b -> p (a b)")
            fv = small.tile([128, 16], mybir.dt.bfloat16, tag="fv")
            fi = small.tile([128, 16], mybir.dt.uint32, tag="fi")
            ic2 = small.tile([128, 16 * L], mybir.dt.bfloat16, tag="ic2")
            nc.vector.max(fv[:, 0:8], picf)
            nc.vector.max_index(fi[:, 0:8], fv[:, 0:8], picf)
            nc.vector.match_replace(ic2[:], fv[:, 0:8], picf, -3.0e38)
            nc.vector.max(fv[:, 8:16], ic2[:])
            nc.vector.max_index(fi[:, 8:16], fv[:, 8:16], ic2[:])
            nc.sync.dma_start(og.ap()[pt * QT:(pt + 1) * QT, :], pgi[:])
            nc.sync.dma_start(of.ap()[pt * QT:(pt + 1) * QT, :], fi[:])

        for t in [tq for _ in range(repeat) for tq in range(NQT)]:
            sb = sims[t % NSIMS]
            spill = spills[t]

            bm = small.tile([128, G], mybir.dt.bfloat16, tag="bm")
            c0 = 0
            while c0 < NLOC:
                cl = min(CHUNK, NLOC - c0)
                ps = psum.tile([128, CHUNK], mybir.dt.float32)
                for s in range(0, cl, SUB):
                    for h in range(2):
                        nc.tensor.matmul(
                            ps[:, s:s + SUB],
                            qt_sb[:, h, t * QT:(t + 1) * QT],
                            mt_sb[:, h, c0 + s:c0 + s + SUB],
                            start=(h == 0), stop=(h == 1),
                        )
                # PSUM -> bf16 sims plane (contiguous)
                nc.scalar.copy(sb[:, c0:c0 + cl], ps[:, :cl])
                # spill this chunk to DRAM (row q*G+g of L bf16)
                nc.sync.dma_start(
                    spill[:].rearrange("r l -> (r l)").rearrange(
                        "(q n) -> q n", q=QT)[:, c0:c0 + cl],
                    sb[:, c0:c0 + cl],
                )
                # grouped max scan of this chunk
                nc.vector.reduce_max(
                    bm[:, c0 // L:(c0 + cl) // L],
                    sb[:, c0:c0 + cl].rearrange("p (g l) -> p g l", l=L),
                    axis=mybir.AxisListType.X,
                )
                c0 += cl
                # flush the previous tile's final extraction mid-loop so the
                # DVE has work while this tile's ACT copies ramp up
                if c0 == 2 * CHUNK and len(pending) > 1:
                    _emit_final(pending.pop(0))

            # top-16 groups (two rounds of 8); gathers for round 1 dispatch
            # while round 2 still runs on the vector engine
            gv = small.tile([128, 16], mybir.dt.bfloat16, tag="gv")
            gi = small.tile([128, 16], mybir.dt.uint32, tag="gi")
            bm2 = small.tile([128, G], mybir.dt.bfloat16, tag="bm2")
            offs = small.tile([128, 16], mybir.dt.uint32, tag="offs")
            ic = small.tile([128, 16, L], mybir.dt.bfloat16, tag="ic")

            nc.vector.max(gv[:, 0:8], bm[:])
            nc.vector.max_index(gi[:, 0:8], gv[:, 0:8], bm[:])
            nc.vector.tensor_tensor(offs[:, 0:8], gi[:, 0:8], qbase[:, 0:8],
                                    mybir.AluOpType.add)
            for j in range(8):
                nc.gpsimd.indirect_dma_start(
                    out=ic[:, j, :], out_offset=None, in_=spill[:],
                    in_offset=bass.IndirectOffsetOnAxis(ap=offs[:, j:j + 1], axis=0),
                )

            nc.vector.match_replace(bm2[:], gv[:, 0:8], bm[:], -3.0e38)
            nc.vector.max(gv[:, 8:16], bm2[:])
            nc.vector.max_index(gi[:, 8:16], gv[:, 8:16], bm2[:])
            nc.vector.tensor_tensor(offs[:, 8:16], gi[:, 8:16], qbase[:, 8:16],
                                    mybir.AluOpType.add)
            for j in range(8, 16):
                nc.gpsimd.indirect_dma_start(
                    out=ic[:, j, :], out_offset=None, in_=spill[:],
                    in_offset=bass.IndirectOffsetOnAxis(ap=offs[:, j:j + 1], axis=0),
                )

            # defer the final extraction of this tile; it is flushed inside
            # the next tile's chunk loop (software pipelining)
            pending.append((t, ic, gi))

        for p in pending:
            _emit_final(p)

    nc.compile()
    return nc


def _get_program():
    if "nc" not in _cache:
        _cache["nc"] = _build_program()
    return _cache["nc"]


def _prep_inputs(query, memory_feat):
    qn = np.sqrt((query.astype(np.float32) ** 2).sum(-1, keepdims=True))
    qhat = query / np.clip(qn, EPS, None)
    mn = np.sqrt((memory_feat.astype(np.float32) ** 2).sum(-1, keepdims=True))
    mhat = memory_feat / np.clip(mn, EPS, None)

    # qt: (128, 2, B) bf16 with qt[p, h, b] = qhat[b, h*128+p]
    qtl = np.ascontiguousarray(
        qhat.T.reshape(2, 128, B).transpose(1, 0, 2)
    ).astype(ml_dtypes.bfloat16)

    # memory shards: (128, 2, NLOC) bf16 with mt[p, h, j] = mhat[c*12500+j, h*128+p]
    mts = []
    for c in range(NCORES):
        slab = mhat[c * NLOC_REAL:(c + 1) * NLOC_REAL]
        slab = np.concatenate(
            [slab, np.zeros((NLOC - NLOC_REAL, D), np.float32)], axis=0
        )
        mtl = np.ascontiguousarray(
            slab.T.reshape(2, 128, NLOC).transpose(1, 0, 2)
        ).astype(ml_dtypes.bfloat16)
        mts.append(mtl)
    return qhat, mhat, qtl, mts


def _fuse_host(topv, topi, memory_evidence, model_evidence):
    """Exact fp32 mirror of the reference softmax + DST fusion."""
    f32 = np.float32
    w = topv.astype(f32) / f32(TEMPERATURE)
    w = w - w.max(-1, keepdims=True)
    w = np.exp(w)
    w = w / w.sum(-1, keepdims=True)

    ev = memory_evidence[topi]                      # (B, k, K)
    alpha_r = f32(1.0) + np.einsum("bk,bkc->bc", w, ev.astype(f32))
    alpha_m = model_evidence.astype(f32) + f32(1.0)

    def alpha_to_belief_u(alpha):
        Kd = alpha.shape[-1]
        S = np.clip(alpha.sum(-1, keepdims=True), EPS, None)
        b = np.clip((alpha - 1.0) / S, 0.0, None)
        u = np.clip(Kd / S, EPS, 1.0 - EPS)
        b_sum = b.sum(-1, keepdims=True)
        target = np.clip(1.0 - u, EPS, None)
        b = b * (target / np.clip(b_sum, EPS, None))
        return b.astype(f32), u.astype(f32)

    def combine_two_opinions(b1, u1, b2, u2):
        total_pair = b1.sum(-1, keepdims=True) * b2.sum(-1, keepdims=True)
        dot_same = (b1 * b2).sum(-1, keepdims=True)
        C = total_pair - dot_same
        S = np.clip(1.0 - C, EPS, None)
        b = (b1 * b2 + b1 * u2 + b2 * u1) / S
        u = u1 * u2 / S
        b = np.clip(b, 0.0, None)
        u = np.clip(u, EPS, 1.0 - EPS)
        b_sum = b.sum(-1, keepdims=True)
        b = b * ((1.0 - u) / np.clip(b_sum, EPS, None))
        return b.astype(f32), u.astype(f32)

    def opinion_to_alpha(b, u):
        Kd = b.shape[-1]
        u = np.clip(u, EPS, 1.0 - EPS)
        S = Kd / u
        alpha = b * S + 1.0
        return np.clip(alpha, 1.0 + EPS, None).astype(f32)

    b_m, u_m = alpha_to_belief_u(alpha_m)
    b_r, u_r = alpha_to_belief_u(alpha_r)
    b_f, u_f = combine_two_opinions(b_m, u_m, b_r, u_r)
    return opinion_to_alpha(b_f, u_f)


def kernel(query, memory_feat, memory_evidence, model_evidence, top_k):
    top_k = int(top_k)
    assert top_k == TOPK

    query = np.asarray(query, dtype=np.float32)
    memory_feat = np.asarray(memory_feat, dtype=np.float32)
    memory_evidence = np.asarray(memory_evidence, dtype=np.float32)
    model_evidence = np.asarray(model_evidence, dtype=np.float32)

    nc = _get_program()
    qhat, mhat, qtl, mts = _prep_inputs(query, memory_feat)

    in_maps = [{"mt": mts[c], "qt": qtl} for c in range(NCORES)]
    res = bass_utils.run_bass_kernel_spmd(nc, in_maps, core_ids=list(range(NCORES)))
    _cache["last_results"] = res

    # compose the two index levels: global candidate index per (core, q, slot)
    cand_idx = np.empty((B, NCORES * TOPK), dtype=np.int64)
    for c in range(NCORES):
        gids = res.results[c]["og"].astype(np.int64)    # (B,16) group ids in [0,G)
        fidx = res.results[c]["of"].astype(np.int64)    # (B,16) positions in [0,512)
        j = fidx >> 5                                    # which gathered slot
        r = fidx & 31                                    # position within group
        grp = np.take_along_axis(gids, j, axis=1)        # group id per candidate
        pos = grp * L + r                                # position in the slab
        valid = pos < NLOC_REAL
        gidx = c * NLOC_REAL + np.clip(pos, 0, NLOC_REAL - 1)
        gidx[~valid] = -1
        cand_idx[:, c * TOPK:(c + 1) * TOPK] = gidx

    # exact fp32 rescore of the 128 candidates per query
    safe_idx = np.clip(cand_idx, 0, N - 1)
    mh_c = mhat[safe_idx]                                # (B, 128, D)
    s = np.einsum("bd,bkd->bk", qhat, mh_c).astype(np.float32)
    s[cand_idx < 0] = -np.inf

    order = np.argsort(-s, axis=1, kind="stable")[:, :TOPK]
    topv = np.take_along_axis(s, order, axis=1)
    topi = np.take_along_axis(cand_idx, order, axis=1)

    return _fuse_host(topv, topi, memory_evidence, model_evidence)
